# revision 39
# baseline (speedup 1.0000x reference)
"""AttentionBlock kernel for 8 Trainium2 NeuronCores.

Sharding: one (batch, head) pair per core (B=2 x H=4 = 8 cores).
Per core, for its (b, h):
    qT/kT = (w_q/k^T @ x_b) + bias            [64, S] fp16  (S pad 2816)
    scores S^T[j, i] = sum_d k[j,d] q[i,d]    fp16 matmuls, fp32 PSUM.
        The two j-tiles of a group run CONCURRENTLY via PE row tiling:
        even tile on array rows 0-63 (kT_sb/qT_sb parts 0-63), odd tile
        on rows 64-127 (kT_hi/qT_sb parts 64-127, filled by SBUF->SBUF
        DMA dup after the projection drains). tile_position auto-derives
        from base_partition; both planes drain to different PSUM banks.
    P = exp(S^T * 0.125 - 3)  -> fp8e4m3, whole groups alternate engine:
        even g: ScalarE native exp (fp8 out), one [128,2,iw] ACTIVATE
        odd  g: DVE Schraudolph: u8 = rint(s*(8*0.125/ln2)+21.03),
                bitcast to e4m3 (exact 2^x byte encode, +-3% ripple;
                softmax normalization cancels the mean bias)
    resT[d, i] = sum_j v8[j, d] P[j, i]       fp8 DoubleRow matmuls: two
        j-tiles (256 rows) per instruction at 0.5 cycles/col; v8 is fp8
        [128, 2, 128] with col 64 = ones (softmax denominator l in row 64),
        cols 65..127 zero (dual-fp8 ldweights requires M=128).
    outT[c, i] = sum_d w_out[d, c] resT[d, i] fp16, fp32 PSUM -> fp16 SBUF
Host: out_b = sum_h (outT / l + (b_v @ w_out_h)) + b_out + x_b.

Steady state target ~650ns per 2-j-tile group: PE issues the score pair
concurrently (~230ns span) + PV-DR (~216ns); the exp instruction
(ScalarE ~1.0us / DVE ~1.2us per [128,1024]) alternates engines so each
covers every other group. Pool cannot access PSUM so all drains + exp
stay on ScalarE/DVE. PV queue is carried across iblock boundaries with
lag 3 (the PE queue is in-order; an early PV waiting on its exp would
stall the score matmuls behind it), and the ex pool is sized for that
lag. Head: small weight DMAs issue before x so the first qk chunk is
not starved behind the 1.4MB x stream.
"""

import numpy as np

C = 256
S = 2744
SP = 2816  # 22 * 128
H = 4
DK = 64
NT = 22  # j tiles of 128
NG = 11  # groups of 2 j-tiles
SVALID_LAST = S - 21 * 128  # 56 valid rows in last j-tile

IBLOCKS = [(0, 512), (512, 512), (1024, 512), (1536, 512), (2048, 512), (2560, 184)]

LN2 = float(np.log(2.0))
# Schraudolph byte encode: u8 = rint(s * SCHRAU_SCALE + SCHRAU_BIAS)
SCHRAU_SCALE = 0.125 * 8.0 / LN2
SCHRAU_BIAS = 56.0 - 24.0 / LN2 - 0.344  # -0.344 centers ripple vs exact lane

_NC = None


def _build():
    from contextlib import ExitStack

    import concourse.bacc as bacc
    import concourse.tile as tile
    from concourse import mybir

    f32 = mybir.dt.float32
    f16 = mybir.dt.float16
    f8 = mybir.dt.float8e4
    u8 = mybir.dt.uint8
    Exp = mybir.ActivationFunctionType.Exp
    DR = mybir.MatmulPerfMode.DoubleRow
    Mult = mybir.AluOpType.mult
    Add = mybir.AluOpType.add

    nc = bacc.Bacc("TRN2", target_bir_lowering=False)

    xT = nc.dram_tensor("xT", [C, S], f16, kind="ExternalInput")
    w3 = nc.dram_tensor("w3", [C, 3 * DK], f16, kind="ExternalInput")
    bqk2 = nc.dram_tensor("bqk2", [128, 1], f32, kind="ExternalInput")
    ones_tail = nc.dram_tensor("ones_tail", [128, 1], f8, kind="ExternalInput")

    # res rows 0:64 = unnormalized attention output resT, row 64 = softmax
    # denominator l. The cheap [64,C] output projection + normalization +
    # residual happen host-side in f32 (alongside the existing lsum divide),
    # which removes the per-iblock out-proj matmuls, PSUM->SBUF tail copies
    # and the 4x larger out DMA from the device critical path.
    res_d = nc.dram_tensor("res", [DK + 1, S], f16, kind="ExternalOutput")

    with tile.TileContext(nc) as tc, ExitStack() as ctx:
        consts = ctx.enter_context(tc.tile_pool(name="consts", bufs=1))
        big = ctx.enter_context(tc.tile_pool(name="big", bufs=1))
        expp = ctx.enter_context(tc.tile_pool(name="expp", bufs=9))
        resp = ctx.enter_context(tc.tile_pool(name="resp", bufs=3))
        # PSUM 8 banks: scp 3x[128,1024]f32 (6 banks) shared by score pairs
        # AND the projection chunks; T-pool 2x[128,512] (1 bank each) holds
        # the PV accumulation, double-buffered so the next iblock's PV
        # stream starts without waiting the prev T's res drain.
        scp = ctx.enter_context(tc.tile_pool(name="scp", bufs=3, space="PSUM"))
        tp = ctx.enter_context(tc.tile_pool(name="tp", bufs=2, space="PSUM"))

        # ---- weights / constants ----
        # w3 first on gpsimd (SWDGE ~1us latency but nothing else queued);
        # the two HWDGE queues are reserved for the x stream
        w_sb = consts.tile([128, 2, 3 * DK], f16)
        nc.gpsimd.dma_start(
            out=w_sb, in_=w3.rearrange("(c p) d -> p c d", p=128)
        )

        def wslice(idx, cc):
            return w_sb[:, cc, idx * DK : (idx + 1) * DK]

        # combined per-partition bias: rows 0:64 = b_q, rows 64:128 = b_k
        # (matches the fused q/k projection drain layout)
        b2_sb = consts.tile([128, 1], f32)
        nc.gpsimd.dma_start(out=b2_sb, in_=bqk2[:, :])
        ebias_sb = consts.tile([128, 1], f32)
        nc.vector.memset(ebias_sb, -3.0)

        # ---- HAM warm-up: dummy matmuls on a zeroed scratch tile while
        # the x DMA streams in. The PE sits idle ~5us waiting for x; these
        # keep it busy so the 4096-cycle activity window unthrottles the
        # clock (1.2 -> 2.4 GHz) before the real phase-A matmuls start.
        warm_sb = consts.tile([128, 512], f16)
        nc.gpsimd.memset(warm_sb, 0.0)
        warm_ps = scp.tile([128, 1024], f32, tag="sc", name="warm_ps")
        for wi in range(10):
            nc.tensor.matmul(
                warm_ps[:, :512],
                lhsT=warm_sb[:, :128],
                rhs=warm_sb,
                start=True,
                stop=True,
            )

        # ---- x in SBUF (split DMA: small first chunk unblocks phase A) --
        x_sb = big.tile([128, 2, SP], f16)
        nc.gpsimd.memset(x_sb[:, :, S:SP], 0.0)
        xr = xT.rearrange("(c p) s -> p c s", p=128)
        # spread x over the two DMA-capable HWDGE queues (sync + scalar):
        # one queue alone sustains only ~140GB/s on this pattern, and each
        # chunk costs its full 256 descriptors regardless of width
        nc.sync.dma_start(out=x_sb[:, :, :512], in_=xr[:, :, :512])
        nc.scalar.dma_start(out=x_sb[:, :, 512:1024], in_=xr[:, :, 512:1024])
        nc.sync.dma_start(out=x_sb[:, :, 1024:2048], in_=xr[:, :, 1024:2048])
        nc.scalar.dma_start(out=x_sb[:, :, 2048:S], in_=xr[:, :, 2048:])

        # ---- persistent big tiles ----
        # qk_sb: fused projection drain target — q on parts 0-63, k on
        # parts 64-127 (the q and k matmuls run concurrently on array col
        # groups 0/64 sharing the x stream, and drain in ONE instruction).
        # MM_B (odd j-tiles) uses qk_sb[64:128] as lhsT directly; MM_A
        # needs the even k tiles dup'd down to parts 0-63 (k_lo) and the q
        # rhs dup'd up to parts 64-127 (q_hi).
        qk_sb = big.tile([128, SP], f16)
        q_hi = big.tile([128, SP], f16)  # rows 64:128 = q copy
        k_lo = big.tile([DK, NG * 128], f16)  # even j-tiles' k, parts 0-63
        # v8: [j-in-tile, group, plane(u), 128]: cols 0:64 v, 64 ones, 65+ 0
        # (dual-fp8 ldweights requires M=128 here; M=96 measured slower)
        v8_sb = big.tile([128, NG, 2, 128], f8)
        # memsets on Pool: only cols 64: need zeroing (v drains cover 0:64
        # with matmul zeros in padded rows); keeps the Pool queue short so
        # the kT/qT dup DMAs land early
        nc.gpsimd.memset(v8_sb[:, :, :, DK:], 0.0)
        nc.gpsimd.memset(v8_sb[:, :, :, DK : DK + 1], 1.0)
        # last j-tile (group 10, plane 1) has only SVALID_LAST valid rows;
        # partition-base-56 memset is illegal, so DMA a host mask instead
        nc.gpsimd.dma_start(
            out=v8_sb[:, NG - 1, 1, DK : DK + 1], in_=ones_tail[:, :]
        )

        Identity = mybir.ActivationFunctionType.Identity
        QBLOCKS = [(0, 1024), (1024, 1024), (2048, 768)]

        # ---- fused q+k projection chunk (1024-col blocks) ----
        # q into array col group 0 (PSUM parts 0:64) and k into col group
        # 64 (parts 64:128) run CONCURRENTLY on the same x chunk; one
        # [128, w] drain covers both at the cost of a [64, w] one.
        def kq_chunk(qb, eng):
            off, w = QBLOCKS[qb]
            ps = scp.tile([128, 1024], f32, tag="sc", name="psqk")
            for half in range(0, w, 512):
                hw_ = min(512, w - half)
                for cc in range(2):
                    for widx, p0 in ((0, 0), (1, DK)):
                        nc.tensor.matmul(
                            ps[p0 : p0 + DK, half : half + hw_],
                            lhsT=wslice(widx, cc),
                            rhs=x_sb[:, cc, off + half : off + half + hw_],
                            start=(cc == 0),
                            stop=(cc == 1),
                        )
            if eng is nc.scalar:
                nc.scalar.activation(
                    out=qk_sb[:, off : off + w], in_=ps[:, :w],
                    func=Identity, bias=b2_sb,
                )
            else:
                eng.tensor_scalar(
                    qk_sb[:, off : off + w], ps[:, :w], b2_sb, None,
                    mybir.AluOpType.add,
                )

        # dup DMAs (Pool queue) after each chunk's drain: q up to parts
        # 64-127 (rhs of MM_B), even k tiles down to parts 0-63 (lhsT of
        # MM_A)
        def dup_q(qb):
            off, w = QBLOCKS[qb]
            nc.gpsimd.dma_start(
                out=q_hi[64:128, off : off + w], in_=qk_sb[0:64, off : off + w]
            )

        def dup_k(qb):
            off, w = QBLOCKS[qb]
            nt2 = w // 256  # even tiles in this block
            src = qk_sb[64:128, off : off + w].rearrange(
                "p (t u c) -> p t u c", u=2, c=128
            )[:, :, 0, :]
            dst = k_lo[
                :, (off // 256) * 128 : (off // 256) * 128 + nt2 * 128
            ].rearrange("p (t c) -> p t c", c=128)
            nc.gpsimd.dma_start(out=dst, in_=src)

        # ---- v projection chunk: 4 j-tiles -> v8 groups 2c, 2c+1 ----
        def v_chunk(c, eng):
            nt = min(4, NT - 4 * c)
            ps = scp.tile([128, 1024], f32, tag="sc", name="psv")
            pv4 = ps[:, : nt * DK].rearrange("p (t d) -> p t d", t=nt)
            for ti in range(nt):
                t = 4 * c + ti
                for cc in range(2):
                    nc.tensor.matmul(
                        pv4[:, ti, :],
                        lhsT=x_sb[:, cc, t * 128 : (t + 1) * 128],
                        rhs=wslice(2, cc),
                        start=(cc == 0),
                        stop=(cc == 1),
                    )
            dst = v8_sb[:, 2 * c : 2 * c + (nt + 1) // 2, :, :DK]
            eng.tensor_copy(dst, pv4)

        # phase A: all three fused k+q chunks, v chunk 0. Deliberately
        # serial-ish: it doubles as a power ramp — slamming straight into
        # the main loop from cold measured bimodal (88us cold-start vs
        # 104us when the previous run depleted the power-throttle budget).
        kq_chunk(0, nc.scalar)
        dup_k(0)
        dup_q(0)
        kq_chunk(1, nc.vector)
        dup_k(1)
        dup_q(1)
        kq_chunk(2, nc.scalar)
        dup_k(2)
        dup_q(2)
        v_chunk(0, nc.vector)

        # v chunks interleaved into iblock 0 (key: (ibi, g) -> emit fn)
        ib_chunks = {
            (0, 1): lambda: v_chunk(1, nc.vector),
            (0, 3): lambda: v_chunk(2, nc.vector),
            (0, 5): lambda: v_chunk(3, nc.vector),
            (0, 7): lambda: v_chunk(4, nc.vector),
            (0, 9): lambda: v_chunk(5, nc.vector),
        }

        # ---- main attention loop ----
        # PV queue lag 3 and carried across iblock boundaries: the PE queue
        # is in-order, so a PV emitted right after its exp stalls the score
        # matmuls behind it; with lag 3 the exp has long completed.
        def emit_pv(pv, pex, pg, iw):
            if iw >= 384:
                nc.tensor.matmul(
                    pv[:, :iw],
                    lhsT=v8_sb[:, pg, :, :],
                    rhs=pex[:, :, :iw],
                    start=(pg == 0),
                    stop=(pg == NG - 1),
                    perf_mode=DR,
                )
            else:
                # narrow iblock: DoubleRow's 256-col ldweights (~213ns)
                # dwarfs the stream; two plain fp8 matmuls with FWL
                # (~27ns ldw each) are faster
                for u in range(2):
                    nc.tensor.matmul(
                        pv[:, :iw],
                        lhsT=v8_sb[:, pg, u, :],
                        rhs=pex[:, u, :iw],
                        start=(pg == 0 and u == 0),
                        stop=(pg == NG - 1 and u == 1),
                    )

        def res_store(ibi, pv, ioff, iw):
            res_sb = resp.tile([DK + 1, 512], f16, tag="res", name="res_sb")
            # one DVE instruction: ScalarE is the tighter exp engine
            nc.vector.tensor_copy(res_sb[:, :iw], pv[: DK + 1, :iw])
            nc.sync.dma_start(
                out=res_d[:, ioff : ioff + iw], in_=res_sb[:, :iw]
            )

        pvq = []  # [(ex, g, ibi)]
        state = {"T": None, "prev": None}  # prev: (ibi, pv, ioff, iw)

        def pop_pv():
            ex, g, ibi_ = pvq.pop(0)
            ioff_, iw_ = IBLOCKS[ibi_]
            emit_pv(state["T"], ex, g, iw_)

        for ibi, (ioff, iw) in enumerate(IBLOCKS):
            for g in range(NG):
                # pop PVs BEFORE this group's score matmuls: the PV
                # stream hides the next matmul's ldweights (in-order PE
                # queue + one-deep weight shadow slot)
                if g == 3 and ibi > 0:
                    pop_pv()  # PV(10, prev); pv(prev) now complete
                    pibi, ppv, pioff, piw = state["prev"]
                    res_store(pibi, ppv, pioff, piw)
                elif g == 4:
                    # double-buffered T: the new PV stream starts without
                    # waiting for the prev T's res drain
                    state["T"] = tp.tile([128, 512], f32, tag="T", name="T")
                    pop_pv()  # first PV of this iblock, into the new T
                elif len(pvq) >= 4:
                    pop_pv()
                if (ibi, g) in ib_chunks:
                    ib_chunks[(ibi, g)]()
                sc = scp.tile([128, 1024], f32, tag="sc", name="sc")
                sc3 = sc.rearrange("p (b w) -> p b w", b=2)[:, :, :iw]
                # score pair: even j-tile on array rows 0-63, odd j-tile on
                # rows 64-127 (tile_position from base_partition 64) -> the
                # two matmuls run concurrently in the PE array
                nc.tensor.matmul(
                    sc3[:, 0, :],
                    lhsT=k_lo[:, g * 128 : (g + 1) * 128],
                    rhs=qk_sb[0:DK, ioff : ioff + iw],
                    start=True,
                    stop=True,
                )
                nc.tensor.matmul(
                    sc3[:, 1, :],
                    lhsT=qk_sb[64:128, (2 * g + 1) * 128 : (2 * g + 2) * 128],
                    rhs=q_hi[64:128, ioff : ioff + iw],
                    start=True,
                    stop=True,
                )
                ex = expp.tile([128, 2, 512], f8, tag="ex", name="ex")
                # whole-group exp, alternating engines: even g ScalarE
                # (native exp), odd g DVE (Schraudolph byte encode)
                if g % 2 == 0:
                    nc.scalar.activation(
                        out=ex[:, :, :iw],
                        in_=sc3,
                        func=Exp,
                        bias=ebias_sb,
                        scale=0.125,
                    )
                else:
                    nc.vector.tensor_scalar(
                        ex[:, :, :iw].bitcast(u8), sc3,
                        SCHRAU_SCALE, SCHRAU_BIAS, Mult, Add,
                    )
                pvq.append((ex, g, ibi))
            state["prev"] = (ibi, state["T"], ioff, iw)
        while pvq:
            pop_pv()
        pibi, ppv, pioff, piw = state["prev"]
        res_store(pibi, ppv, pioff, piw)

    nc.compile()
    return nc


def _get_nc():
    global _NC
    if _NC is None:
        _NC = _build()
    return _NC


def _ones_tail():
    import ml_dtypes

    m = np.zeros((128, 1), dtype=ml_dtypes.float8_e4m3)
    m[:SVALID_LAST] = 1.0
    return m


def _make_in_maps(inputs):
    x = np.asarray(inputs["x"], dtype=np.float32)
    w_proj = np.asarray(inputs["w_proj"], dtype=np.float32)
    b_proj = np.asarray(inputs["b_proj"], dtype=np.float32)
    w_out = np.asarray(inputs["w_out"], dtype=np.float32)
    in_maps = []
    for core in range(8):
        b, h = divmod(core, H)
        base = h * 3 * DK
        in_maps.append(
            {
                "xT": np.ascontiguousarray(x[b].reshape(C, S).astype(np.float16)),
                "w3": np.ascontiguousarray(
                    w_proj[:, base : base + 3 * DK].astype(np.float16)
                ),
                "bqk2": np.ascontiguousarray(
                    np.concatenate(
                        [
                            b_proj[base : base + DK],
                            b_proj[base + DK : base + 2 * DK],
                        ]
                    ).astype(np.float32)[:, None]
                ),
                "ones_tail": _ones_tail(),
            }
        )
    return in_maps


def kernel(x, w_proj, b_proj, w_out, b_out):
    from concourse.bass_utils import run_bass_kernel_spmd

    x = np.asarray(x, dtype=np.float32)
    w_proj = np.asarray(w_proj, dtype=np.float32)
    b_proj = np.asarray(b_proj, dtype=np.float32)
    w_out = np.asarray(w_out, dtype=np.float32)
    b_out = np.asarray(b_out, dtype=np.float32)

    B = x.shape[0]
    nc = _get_nc()
    in_maps = _make_in_maps(
        {"x": x, "w_proj": w_proj, "b_proj": b_proj, "w_out": w_out, "b_out": b_out}
    )
    res = run_bass_kernel_spmd(nc, in_maps, list(range(8)))

    outs = np.zeros((B, C, S), dtype=np.float32)
    for b in range(B):
        acc = x[b].reshape(C, S).astype(np.float32) + b_out[:, None]
        for h in range(H):
            core = b * H + h
            r65 = res.results[core]["res"].astype(np.float32)  # [65, S]
            rn = r65[:DK] / r65[DK : DK + 1]  # softmax-normalized resT [64, S]
            woh = w_out[h * DK : (h + 1) * DK, :]  # [64, C]
            bv = b_proj[h * 3 * DK + 2 * DK : h * 3 * DK + 3 * DK]
            corr = bv @ woh  # [C]
            acc = acc + woh.T @ rn + corr[:, None]
        outs[b] = acc
    return outs.reshape(B, C, 14, 14, 14)


# revision 40
# speedup vs baseline: 1.2273x; 1.2273x over previous
"""AttentionBlock kernel for 8 Trainium2 NeuronCores.

Sharding: one (batch, head) pair per core (B=2 x H=4 = 8 cores).
Per core, for its (b, h):
    qT/kT = (w_q/k^T @ x_b) + bias            [64, S] fp16  (S pad 2816)
    scores S^T[j, i] = sum_d k[j,d] q[i,d]    fp16 matmuls, fp32 PSUM.
        The two j-tiles of a group run CONCURRENTLY via PE row tiling:
        even tile on array rows 0-63 (kT_sb/qT_sb parts 0-63), odd tile
        on rows 64-127 (kT_hi/qT_sb parts 64-127, filled by SBUF->SBUF
        DMA dup after the projection drains). tile_position auto-derives
        from base_partition; both planes drain to different PSUM banks.
    P = exp(S^T * 0.125 - 3)  -> fp8e4m3, whole groups alternate engine:
        even g: ScalarE native exp (fp8 out), one [128,2,iw] ACTIVATE
        odd  g: DVE Schraudolph: u8 = rint(s*(8*0.125/ln2)+21.03),
                bitcast to e4m3 (exact 2^x byte encode, +-3% ripple;
                softmax normalization cancels the mean bias)
    resT[d, i] = sum_j v8[j, d] P[j, i]       fp8 DoubleRow matmuls: two
        j-tiles (256 rows) per instruction at 0.5 cycles/col; v8 is fp8
        [128, 2, 128] with col 64 = ones (softmax denominator l in row 64),
        cols 65..127 zero (dual-fp8 ldweights requires M=128).
    outT[c, i] = sum_d w_out[d, c] resT[d, i] fp16, fp32 PSUM -> fp16 SBUF
Host: out_b = sum_h (outT / l + (b_v @ w_out_h)) + b_out + x_b.

Steady state target ~650ns per 2-j-tile group: PE issues the score pair
concurrently (~230ns span) + PV-DR (~216ns); the exp instruction
(ScalarE ~1.0us / DVE ~1.2us per [128,1024]) alternates engines so each
covers every other group. Pool cannot access PSUM so all drains + exp
stay on ScalarE/DVE. PV queue is carried across iblock boundaries with
lag 3 (the PE queue is in-order; an early PV waiting on its exp would
stall the score matmuls behind it), and the ex pool is sized for that
lag. Head: small weight DMAs issue before x so the first qk chunk is
not starved behind the 1.4MB x stream.
"""

import numpy as np

C = 256
S = 2744
SP = 2816  # 22 * 128
H = 4
DK = 64
NT = 22  # j tiles of 128
NG = 11  # groups of 2 j-tiles
SVALID_LAST = S - 21 * 128  # 56 valid rows in last j-tile

IBLOCKS = [(0, 512), (512, 512), (1024, 512), (1536, 512), (2048, 512), (2560, 184)]

LN2 = float(np.log(2.0))
# Schraudolph byte encode: u8 = rint(s * SCHRAU_SCALE + SCHRAU_BIAS)
SCHRAU_SCALE = 0.125 * 8.0 / LN2
SCHRAU_BIAS = 56.0 - 24.0 / LN2 - 0.344  # -0.344 centers ripple vs exact lane

_NC = None


def _build():
    from contextlib import ExitStack

    import concourse.bacc as bacc
    import concourse.tile as tile
    from concourse import mybir

    f32 = mybir.dt.float32
    f16 = mybir.dt.float16
    f8 = mybir.dt.float8e4
    u8 = mybir.dt.uint8
    Exp = mybir.ActivationFunctionType.Exp
    DR = mybir.MatmulPerfMode.DoubleRow
    Mult = mybir.AluOpType.mult
    Add = mybir.AluOpType.add

    nc = bacc.Bacc("TRN2", target_bir_lowering=False)

    xT = nc.dram_tensor("xT", [C, S], f16, kind="ExternalInput")
    w3 = nc.dram_tensor("w3", [C, 3 * DK], f16, kind="ExternalInput")
    bqk2 = nc.dram_tensor("bqk2", [128, 1], f32, kind="ExternalInput")
    ones_tail = nc.dram_tensor("ones_tail", [128, 1], f8, kind="ExternalInput")

    # res rows 0:64 = unnormalized attention output resT, row 64 = softmax
    # denominator l. The cheap [64,C] output projection + normalization +
    # residual happen host-side in f32 (alongside the existing lsum divide),
    # which removes the per-iblock out-proj matmuls, PSUM->SBUF tail copies
    # and the 4x larger out DMA from the device critical path.
    res_d = nc.dram_tensor("res", [DK + 1, S], f16, kind="ExternalOutput")

    with tile.TileContext(nc) as tc, ExitStack() as ctx:
        consts = ctx.enter_context(tc.tile_pool(name="consts", bufs=1))
        big = ctx.enter_context(tc.tile_pool(name="big", bufs=1))
        expp = ctx.enter_context(tc.tile_pool(name="expp", bufs=9))
        resp = ctx.enter_context(tc.tile_pool(name="resp", bufs=3))
        # PSUM 8 banks: scp 3x[128,1024]f32 (6 banks) shared by score pairs
        # AND the projection chunks; T-pool 2x[128,512] (1 bank each) holds
        # the PV accumulation, double-buffered so the next iblock's PV
        # stream starts without waiting the prev T's res drain.
        scp = ctx.enter_context(tc.tile_pool(name="scp", bufs=3, space="PSUM"))
        tp = ctx.enter_context(tc.tile_pool(name="tp", bufs=2, space="PSUM"))

        # ---- weights / constants ----
        # w3 first on gpsimd (SWDGE ~1us latency but nothing else queued);
        # the two HWDGE queues are reserved for the x stream
        w_sb = consts.tile([128, 2, 3 * DK], f16)
        nc.gpsimd.dma_start(
            out=w_sb, in_=w3.rearrange("(c p) d -> p c d", p=128)
        )

        def wslice(idx, cc):
            return w_sb[:, cc, idx * DK : (idx + 1) * DK]

        # combined per-partition bias: rows 0:64 = b_q, rows 64:128 = b_k
        # (matches the fused q/k projection drain layout)
        b2_sb = consts.tile([128, 1], f32)
        nc.gpsimd.dma_start(out=b2_sb, in_=bqk2[:, :])
        ebias_sb = consts.tile([128, 1], f32)
        nc.vector.memset(ebias_sb, -3.0)

        # ---- x in SBUF (split DMA: small first chunk unblocks phase A) --
        x_sb = big.tile([128, 2, SP], f16)
        nc.gpsimd.memset(x_sb[:, :, S:SP], 0.0)
        xr = xT.rearrange("(c p) s -> p c s", p=128)
        # spread x over the two DMA-capable HWDGE queues (sync + scalar):
        # one queue alone sustains only ~140GB/s on this pattern, and each
        # chunk costs its full 256 descriptors regardless of width
        nc.sync.dma_start(out=x_sb[:, :, :512], in_=xr[:, :, :512])
        nc.scalar.dma_start(out=x_sb[:, :, 512:1024], in_=xr[:, :, 512:1024])
        nc.sync.dma_start(out=x_sb[:, :, 1024:2048], in_=xr[:, :, 1024:2048])
        nc.scalar.dma_start(out=x_sb[:, :, 2048:S], in_=xr[:, :, 2048:])

        # ---- persistent big tiles ----
        # qk_sb: fused projection drain target — q on parts 0-63, k on
        # parts 64-127 (the q and k matmuls run concurrently on array col
        # groups 0/64 sharing the x stream, and drain in ONE instruction).
        # MM_B (odd j-tiles) uses qk_sb[64:128] as lhsT directly; MM_A
        # needs the even k tiles dup'd down to parts 0-63 (k_lo) and the q
        # rhs dup'd up to parts 64-127 (q_hi).
        qk_sb = big.tile([128, SP], f16)
        q_hi = big.tile([128, SP], f16)  # rows 64:128 = q copy
        k_lo = big.tile([DK, NG * 128], f16)  # even j-tiles' k, parts 0-63
        # v8: [j-in-tile, group, plane(u), 128]: cols 0:64 v, 64 ones, 65+ 0
        # (dual-fp8 ldweights requires M=128 here; M=96 measured slower)
        v8_sb = big.tile([128, NG, 2, 128], f8)
        # memsets on Pool: only cols 64: need zeroing (v drains cover 0:64
        # with matmul zeros in padded rows); keeps the Pool queue short so
        # the kT/qT dup DMAs land early
        nc.gpsimd.memset(v8_sb[:, :, :, DK:], 0.0)
        nc.gpsimd.memset(v8_sb[:, :, :, DK : DK + 1], 1.0)
        # last j-tile (group 10, plane 1) has only SVALID_LAST valid rows;
        # partition-base-56 memset is illegal, so DMA a host mask instead
        nc.gpsimd.dma_start(
            out=v8_sb[:, NG - 1, 1, DK : DK + 1], in_=ones_tail[:, :]
        )

        Identity = mybir.ActivationFunctionType.Identity
        QBLOCKS = [(0, 1024), (1024, 1024), (2048, 768)]

        # ---- fused q+k projection chunk (1024-col blocks) ----
        # q into array col group 0 (PSUM parts 0:64) and k into col group
        # 64 (parts 64:128) run CONCURRENTLY on the same x chunk; one
        # [128, w] drain covers both at the cost of a [64, w] one.
        def kq_chunk(qb, eng):
            off, w = QBLOCKS[qb]
            ps = scp.tile([128, 1024], f32, tag="sc", name="psqk")
            for half in range(0, w, 512):
                hw_ = min(512, w - half)
                for cc in range(2):
                    for widx, p0 in ((0, 0), (1, DK)):
                        nc.tensor.matmul(
                            ps[p0 : p0 + DK, half : half + hw_],
                            lhsT=wslice(widx, cc),
                            rhs=x_sb[:, cc, off + half : off + half + hw_],
                            start=(cc == 0),
                            stop=(cc == 1),
                        )
            if eng is nc.scalar:
                nc.scalar.activation(
                    out=qk_sb[:, off : off + w], in_=ps[:, :w],
                    func=Identity, bias=b2_sb,
                )
            else:
                eng.tensor_scalar(
                    qk_sb[:, off : off + w], ps[:, :w], b2_sb, None,
                    mybir.AluOpType.add,
                )

        # dup DMAs (Pool queue) after each chunk's drain: q up to parts
        # 64-127 (rhs of MM_B), even k tiles down to parts 0-63 (lhsT of
        # MM_A)
        def dup_q(qb):
            off, w = QBLOCKS[qb]
            nc.gpsimd.dma_start(
                out=q_hi[64:128, off : off + w], in_=qk_sb[0:64, off : off + w]
            )

        def dup_k(qb):
            off, w = QBLOCKS[qb]
            nt2 = w // 256  # even tiles in this block
            src = qk_sb[64:128, off : off + w].rearrange(
                "p (t u c) -> p t u c", u=2, c=128
            )[:, :, 0, :]
            dst = k_lo[
                :, (off // 256) * 128 : (off // 256) * 128 + nt2 * 128
            ].rearrange("p (t c) -> p t c", c=128)
            nc.gpsimd.dma_start(out=dst, in_=src)

        # ---- v projection chunk: 4 j-tiles -> v8 groups 2c, 2c+1 ----
        def v_chunk(c, eng):
            nt = min(4, NT - 4 * c)
            ps = scp.tile([128, 1024], f32, tag="sc", name="psv")
            pv4 = ps[:, : nt * DK].rearrange("p (t d) -> p t d", t=nt)
            for ti in range(nt):
                t = 4 * c + ti
                for cc in range(2):
                    nc.tensor.matmul(
                        pv4[:, ti, :],
                        lhsT=x_sb[:, cc, t * 128 : (t + 1) * 128],
                        rhs=wslice(2, cc),
                        start=(cc == 0),
                        stop=(cc == 1),
                    )
            dst = v8_sb[:, 2 * c : 2 * c + (nt + 1) // 2, :, :DK]
            eng.tensor_copy(dst, pv4)

        # phase A: all three fused k+q chunks, v chunk 0. Deliberately
        # serial-ish: it doubles as a power ramp — slamming straight into
        # the main loop from cold measured bimodal (88us cold-start vs
        # 104us when the previous run depleted the power-throttle budget).
        kq_chunk(0, nc.scalar)
        dup_k(0)
        dup_q(0)
        kq_chunk(1, nc.vector)
        dup_k(1)
        dup_q(1)
        kq_chunk(2, nc.scalar)
        dup_k(2)
        dup_q(2)
        v_chunk(0, nc.vector)

        # v chunks interleaved into iblock 0 (key: (ibi, g) -> emit fn)
        ib_chunks = {
            (0, 1): lambda: v_chunk(1, nc.vector),
            (0, 3): lambda: v_chunk(2, nc.vector),
            (0, 5): lambda: v_chunk(3, nc.vector),
            (0, 7): lambda: v_chunk(4, nc.vector),
            (0, 9): lambda: v_chunk(5, nc.vector),
        }

        # ---- main attention loop ----
        # PV queue lag 3 and carried across iblock boundaries: the PE queue
        # is in-order, so a PV emitted right after its exp stalls the score
        # matmuls behind it; with lag 3 the exp has long completed.
        def emit_pv(pv, pex, pg, iw):
            if iw >= 384:
                nc.tensor.matmul(
                    pv[:, :iw],
                    lhsT=v8_sb[:, pg, :, :],
                    rhs=pex[:, :, :iw],
                    start=(pg == 0),
                    stop=(pg == NG - 1),
                    perf_mode=DR,
                )
            else:
                # narrow iblock: DoubleRow's 256-col ldweights (~213ns)
                # dwarfs the stream; two plain fp8 matmuls with FWL
                # (~27ns ldw each) are faster
                for u in range(2):
                    nc.tensor.matmul(
                        pv[:, :iw],
                        lhsT=v8_sb[:, pg, u, :],
                        rhs=pex[:, u, :iw],
                        start=(pg == 0 and u == 0),
                        stop=(pg == NG - 1 and u == 1),
                    )

        def res_store(ibi, pv, ioff, iw):
            res_sb = resp.tile([DK + 1, 512], f16, tag="res", name="res_sb")
            # one DVE instruction: ScalarE is the tighter exp engine
            nc.vector.tensor_copy(res_sb[:, :iw], pv[: DK + 1, :iw])
            nc.sync.dma_start(
                out=res_d[:, ioff : ioff + iw], in_=res_sb[:, :iw]
            )

        pvq = []  # [(ex, g, ibi)]
        state = {"T": None, "prev": None}  # prev: (ibi, pv, ioff, iw)

        def pop_pv():
            ex, g, ibi_ = pvq.pop(0)
            ioff_, iw_ = IBLOCKS[ibi_]
            emit_pv(state["T"], ex, g, iw_)

        for ibi, (ioff, iw) in enumerate(IBLOCKS):
            for g in range(NG):
                # pop PVs BEFORE this group's score matmuls: the PV
                # stream hides the next matmul's ldweights (in-order PE
                # queue + one-deep weight shadow slot)
                if g == 3 and ibi > 0:
                    pop_pv()  # PV(10, prev); pv(prev) now complete
                    pibi, ppv, pioff, piw = state["prev"]
                    res_store(pibi, ppv, pioff, piw)
                elif g == 4:
                    # double-buffered T: the new PV stream starts without
                    # waiting for the prev T's res drain
                    state["T"] = tp.tile([128, 512], f32, tag="T", name="T")
                    pop_pv()  # first PV of this iblock, into the new T
                elif len(pvq) >= 4:
                    pop_pv()
                if (ibi, g) in ib_chunks:
                    ib_chunks[(ibi, g)]()
                sc = scp.tile([128, 1024], f32, tag="sc", name="sc")
                sc3 = sc.rearrange("p (b w) -> p b w", b=2)[:, :, :iw]
                # score pair: even j-tile on array rows 0-63, odd j-tile on
                # rows 64-127 (tile_position from base_partition 64) -> the
                # two matmuls run concurrently in the PE array
                nc.tensor.matmul(
                    sc3[:, 0, :],
                    lhsT=k_lo[:, g * 128 : (g + 1) * 128],
                    rhs=qk_sb[0:DK, ioff : ioff + iw],
                    start=True,
                    stop=True,
                )
                nc.tensor.matmul(
                    sc3[:, 1, :],
                    lhsT=qk_sb[64:128, (2 * g + 1) * 128 : (2 * g + 2) * 128],
                    rhs=q_hi[64:128, ioff : ioff + iw],
                    start=True,
                    stop=True,
                )
                ex = expp.tile([128, 2, 512], f8, tag="ex", name="ex")
                # whole-group exp, alternating engines: even g ScalarE
                # (native exp), odd g DVE (Schraudolph byte encode)
                if g % 2 == 0:
                    nc.scalar.activation(
                        out=ex[:, :, :iw],
                        in_=sc3,
                        func=Exp,
                        bias=ebias_sb,
                        scale=0.125,
                    )
                else:
                    nc.vector.tensor_scalar(
                        ex[:, :, :iw].bitcast(u8), sc3,
                        SCHRAU_SCALE, SCHRAU_BIAS, Mult, Add,
                    )
                pvq.append((ex, g, ibi))
            state["prev"] = (ibi, state["T"], ioff, iw)
        while pvq:
            pop_pv()
        pibi, ppv, pioff, piw = state["prev"]
        res_store(pibi, ppv, pioff, piw)

    nc.compile()
    return nc


def _get_nc():
    global _NC
    if _NC is None:
        _NC = _build()
    return _NC


def _ones_tail():
    import ml_dtypes

    m = np.zeros((128, 1), dtype=ml_dtypes.float8_e4m3)
    m[:SVALID_LAST] = 1.0
    return m


def _make_in_maps(inputs):
    x = np.asarray(inputs["x"], dtype=np.float32)
    w_proj = np.asarray(inputs["w_proj"], dtype=np.float32)
    b_proj = np.asarray(inputs["b_proj"], dtype=np.float32)
    w_out = np.asarray(inputs["w_out"], dtype=np.float32)
    in_maps = []
    for core in range(8):
        b, h = divmod(core, H)
        base = h * 3 * DK
        in_maps.append(
            {
                "xT": np.ascontiguousarray(x[b].reshape(C, S).astype(np.float16)),
                "w3": np.ascontiguousarray(
                    w_proj[:, base : base + 3 * DK].astype(np.float16)
                ),
                "bqk2": np.ascontiguousarray(
                    np.concatenate(
                        [
                            b_proj[base : base + DK],
                            b_proj[base + DK : base + 2 * DK],
                        ]
                    ).astype(np.float32)[:, None]
                ),
                "ones_tail": _ones_tail(),
            }
        )
    return in_maps


def kernel(x, w_proj, b_proj, w_out, b_out):
    from concourse.bass_utils import run_bass_kernel_spmd

    x = np.asarray(x, dtype=np.float32)
    w_proj = np.asarray(w_proj, dtype=np.float32)
    b_proj = np.asarray(b_proj, dtype=np.float32)
    w_out = np.asarray(w_out, dtype=np.float32)
    b_out = np.asarray(b_out, dtype=np.float32)

    B = x.shape[0]
    nc = _get_nc()
    in_maps = _make_in_maps(
        {"x": x, "w_proj": w_proj, "b_proj": b_proj, "w_out": w_out, "b_out": b_out}
    )
    res = run_bass_kernel_spmd(nc, in_maps, list(range(8)))

    outs = np.zeros((B, C, S), dtype=np.float32)
    for b in range(B):
        acc = x[b].reshape(C, S).astype(np.float32) + b_out[:, None]
        for h in range(H):
            core = b * H + h
            r65 = res.results[core]["res"].astype(np.float32)  # [65, S]
            rn = r65[:DK] / r65[DK : DK + 1]  # softmax-normalized resT [64, S]
            woh = w_out[h * DK : (h + 1) * DK, :]  # [64, C]
            bv = b_proj[h * 3 * DK + 2 * DK : h * 3 * DK + 3 * DK]
            corr = bv @ woh  # [C]
            acc = acc + woh.T @ rn + corr[:, None]
        outs[b] = acc
    return outs.reshape(B, C, 14, 14, 14)


# revision 42
# speedup vs baseline: 1.2441x; 1.0137x over previous
"""AttentionBlock kernel for 8 Trainium2 NeuronCores.

Sharding: one (batch, head) pair per core (B=2 x H=4 = 8 cores).
Per core, for its (b, h):
    qT/kT = (w_q/k^T @ x_b) + bias            [64, S] fp16  (S pad 2816)
    scores S^T[j, i] = sum_d k[j,d] q[i,d]    fp16 matmuls, fp32 PSUM.
        The two j-tiles of a group run CONCURRENTLY via PE row tiling:
        even tile on array rows 0-63 (kT_sb/qT_sb parts 0-63), odd tile
        on rows 64-127 (kT_hi/qT_sb parts 64-127, filled by SBUF->SBUF
        DMA dup after the projection drains). tile_position auto-derives
        from base_partition; both planes drain to different PSUM banks.
    P = exp(S^T * 0.125 - 3)  -> fp8e4m3, whole groups alternate engine:
        even g: ScalarE native exp (fp8 out), one [128,2,iw] ACTIVATE
        odd  g: DVE Schraudolph: u8 = rint(s*(8*0.125/ln2)+21.03),
                bitcast to e4m3 (exact 2^x byte encode, +-3% ripple;
                softmax normalization cancels the mean bias)
    resT[d, i] = sum_j v8[j, d] P[j, i]       fp8 DoubleRow matmuls: two
        j-tiles (256 rows) per instruction at 0.5 cycles/col; v8 is fp8
        [128, 2, 128] with col 64 = ones (softmax denominator l in row 64),
        cols 65..127 zero (dual-fp8 ldweights requires M=128).
    outT[c, i] = sum_d w_out[d, c] resT[d, i] fp16, fp32 PSUM -> fp16 SBUF
Host: out_b = sum_h (outT / l + (b_v @ w_out_h)) + b_out + x_b.

Steady state target ~650ns per 2-j-tile group: PE issues the score pair
concurrently (~230ns span) + PV-DR (~216ns); the exp instruction
(ScalarE ~1.0us / DVE ~1.2us per [128,1024]) alternates engines so each
covers every other group. Pool cannot access PSUM so all drains + exp
stay on ScalarE/DVE. PV queue is carried across iblock boundaries with
lag 3 (the PE queue is in-order; an early PV waiting on its exp would
stall the score matmuls behind it), and the ex pool is sized for that
lag. Head: small weight DMAs issue before x so the first qk chunk is
not starved behind the 1.4MB x stream.
"""

import numpy as np

C = 256
S = 2744
SP = 2816  # 22 * 128
H = 4
DK = 64
NT = 22  # j tiles of 128
NG = 11  # groups of 2 j-tiles
SVALID_LAST = S - 21 * 128  # 56 valid rows in last j-tile

IBLOCKS = [(0, 512), (512, 512), (1024, 512), (1536, 512), (2048, 512), (2560, 184)]

LN2 = float(np.log(2.0))
# Schraudolph byte encode: u8 = rint(s * SCHRAU_SCALE + SCHRAU_BIAS)
SCHRAU_SCALE = 0.125 * 8.0 / LN2
SCHRAU_BIAS = 56.0 - 24.0 / LN2 - 0.344  # -0.344 centers ripple vs exact lane

_NC = None


def _build():
    from contextlib import ExitStack

    import concourse.bacc as bacc
    import concourse.tile as tile
    from concourse import mybir

    f32 = mybir.dt.float32
    f16 = mybir.dt.float16
    f8 = mybir.dt.float8e4
    u8 = mybir.dt.uint8
    Exp = mybir.ActivationFunctionType.Exp
    DR = mybir.MatmulPerfMode.DoubleRow
    Mult = mybir.AluOpType.mult
    Add = mybir.AluOpType.add

    nc = bacc.Bacc("TRN2", target_bir_lowering=False)

    xT = nc.dram_tensor("xT", [C, S], f16, kind="ExternalInput")
    w3 = nc.dram_tensor("w3", [C, 3 * DK], f16, kind="ExternalInput")
    bqk2 = nc.dram_tensor("bqk2", [128, 1], f32, kind="ExternalInput")
    ones_tail = nc.dram_tensor("ones_tail", [128, 1], f8, kind="ExternalInput")

    # res rows 0:64 = unnormalized attention output resT, row 64 = softmax
    # denominator l. The cheap [64,C] output projection + normalization +
    # residual happen host-side in f32 (alongside the existing lsum divide),
    # which removes the per-iblock out-proj matmuls, PSUM->SBUF tail copies
    # and the 4x larger out DMA from the device critical path.
    res_d = nc.dram_tensor("res", [DK + 1, S], f16, kind="ExternalOutput")

    with tile.TileContext(nc) as tc, ExitStack() as ctx:
        consts = ctx.enter_context(tc.tile_pool(name="consts", bufs=1))
        big = ctx.enter_context(tc.tile_pool(name="big", bufs=1))
        expp = ctx.enter_context(tc.tile_pool(name="expp", bufs=9))
        resp = ctx.enter_context(tc.tile_pool(name="resp", bufs=3))
        # PSUM 8 banks: scp 3x[128,1024]f32 (6 banks) shared by score pairs
        # AND the projection chunks; T-pool 2x[128,512] (1 bank each) holds
        # the PV accumulation, double-buffered so the next iblock's PV
        # stream starts without waiting the prev T's res drain.
        scp = ctx.enter_context(tc.tile_pool(name="scp", bufs=3, space="PSUM"))
        tp = ctx.enter_context(tc.tile_pool(name="tp", bufs=2, space="PSUM"))

        # ---- weights / constants ----
        # w3 first on gpsimd (SWDGE ~1us latency but nothing else queued);
        # the two HWDGE queues are reserved for the x stream
        w_sb = consts.tile([128, 2, 3 * DK], f16)
        nc.gpsimd.dma_start(
            out=w_sb, in_=w3.rearrange("(c p) d -> p c d", p=128)
        )

        def wslice(idx, cc):
            return w_sb[:, cc, idx * DK : (idx + 1) * DK]

        # combined per-partition bias: rows 0:64 = b_q, rows 64:128 = b_k
        # (matches the fused q/k projection drain layout)
        b2_sb = consts.tile([128, 1], f32)
        nc.gpsimd.dma_start(out=b2_sb, in_=bqk2[:, :])
        ebias_sb = consts.tile([128, 1], f32)
        nc.vector.memset(ebias_sb, -3.0)

        # ---- x in SBUF (split DMA: small first chunk unblocks phase A) --
        x_sb = big.tile([128, 2, SP], f16)
        nc.gpsimd.memset(x_sb[:, :, S:SP], 0.0)
        xr = xT.rearrange("(c p) s -> p c s", p=128)
        # spread x over the two DMA-capable HWDGE queues (sync + scalar):
        # one queue alone sustains only ~140GB/s on this pattern, and each
        # chunk costs its full 256 descriptors regardless of width
        nc.sync.dma_start(out=x_sb[:, :, :512], in_=xr[:, :, :512])
        nc.scalar.dma_start(out=x_sb[:, :, 512:1024], in_=xr[:, :, 512:1024])
        nc.sync.dma_start(out=x_sb[:, :, 1024:2048], in_=xr[:, :, 1024:2048])
        nc.scalar.dma_start(out=x_sb[:, :, 2048:S], in_=xr[:, :, 2048:])

        # ---- persistent big tiles ----
        # qk_sb: fused projection drain target — q on parts 0-63, k on
        # parts 64-127 (the q and k matmuls run concurrently on array col
        # groups 0/64 sharing the x stream, and drain in ONE instruction).
        # MM_B (odd j-tiles) uses qk_sb[64:128] as lhsT directly; MM_A
        # needs the even k tiles dup'd down to parts 0-63 (k_lo) and the q
        # rhs dup'd up to parts 64-127 (q_hi).
        qk_sb = big.tile([128, SP], f16)
        q_hi = big.tile([128, SP], f16)  # rows 64:128 = q copy
        k_lo = big.tile([DK, NG * 128], f16)  # even j-tiles' k, parts 0-63
        # v8: [j-in-tile, group, plane(u), 128]: cols 0:64 v, 64 ones, 65+ 0
        # (dual-fp8 ldweights requires M=128 here; M=96 measured slower)
        v8_sb = big.tile([128, NG, 2, 128], f8)
        # memsets on Pool: only cols 64: need zeroing (v drains cover 0:64
        # with matmul zeros in padded rows); keeps the Pool queue short so
        # the kT/qT dup DMAs land early
        nc.gpsimd.memset(v8_sb[:, :, :, DK:], 0.0)
        nc.gpsimd.memset(v8_sb[:, :, :, DK : DK + 1], 1.0)
        # last j-tile (group 10, plane 1) has only SVALID_LAST valid rows;
        # partition-base-56 memset is illegal, so DMA a host mask instead
        nc.gpsimd.dma_start(
            out=v8_sb[:, NG - 1, 1, DK : DK + 1], in_=ones_tail[:, :]
        )

        Identity = mybir.ActivationFunctionType.Identity
        QBLOCKS = [(0, 1024), (1024, 1024), (2048, 768)]

        # ---- fused q+k projection chunk (1024-col blocks) ----
        # q into array col group 0 (PSUM parts 0:64) and k into col group
        # 64 (parts 64:128) run CONCURRENTLY on the same x chunk; one
        # [128, w] drain covers both at the cost of a [64, w] one.
        def kq_chunk(qb, eng):
            off, w = QBLOCKS[qb]
            ps = scp.tile([128, 1024], f32, tag="sc", name="psqk")
            for half in range(0, w, 512):
                hw_ = min(512, w - half)
                for cc in range(2):
                    for widx, p0 in ((0, 0), (1, DK)):
                        nc.tensor.matmul(
                            ps[p0 : p0 + DK, half : half + hw_],
                            lhsT=wslice(widx, cc),
                            rhs=x_sb[:, cc, off + half : off + half + hw_],
                            start=(cc == 0),
                            stop=(cc == 1),
                        )
            if eng is nc.scalar:
                nc.scalar.activation(
                    out=qk_sb[:, off : off + w], in_=ps[:, :w],
                    func=Identity, bias=b2_sb,
                )
            else:
                eng.tensor_scalar(
                    qk_sb[:, off : off + w], ps[:, :w], b2_sb, None,
                    mybir.AluOpType.add,
                )

        # dup DMAs (Pool queue) after each chunk's drain: q up to parts
        # 64-127 (rhs of MM_B), even k tiles down to parts 0-63 (lhsT of
        # MM_A)
        def dup_q(qb):
            off, w = QBLOCKS[qb]
            nc.gpsimd.dma_start(
                out=q_hi[64:128, off : off + w], in_=qk_sb[0:64, off : off + w]
            )

        def dup_k(qb):
            off, w = QBLOCKS[qb]
            nt2 = w // 256  # even tiles in this block
            src = qk_sb[64:128, off : off + w].rearrange(
                "p (t u c) -> p t u c", u=2, c=128
            )[:, :, 0, :]
            dst = k_lo[
                :, (off // 256) * 128 : (off // 256) * 128 + nt2 * 128
            ].rearrange("p (t c) -> p t c", c=128)
            nc.gpsimd.dma_start(out=dst, in_=src)

        # ---- v projection chunk: 4 j-tiles -> v8 groups 2c, 2c+1 ----
        def v_chunk(c, eng):
            nt = min(4, NT - 4 * c)
            ps = scp.tile([128, 1024], f32, tag="sc", name="psv")
            pv4 = ps[:, : nt * DK].rearrange("p (t d) -> p t d", t=nt)
            for ti in range(nt):
                t = 4 * c + ti
                for cc in range(2):
                    nc.tensor.matmul(
                        pv4[:, ti, :],
                        lhsT=x_sb[:, cc, t * 128 : (t + 1) * 128],
                        rhs=wslice(2, cc),
                        start=(cc == 0),
                        stop=(cc == 1),
                    )
            dst = v8_sb[:, 2 * c : 2 * c + (nt + 1) // 2, :, :DK]
            eng.tensor_copy(dst, pv4)

        # phase A: all three fused k+q chunks, v chunk 0. Deliberately
        # serial-ish: it doubles as a power ramp — slamming straight into
        # the main loop from cold measured bimodal (88us cold-start vs
        # 104us when the previous run depleted the power-throttle budget).
        kq_chunk(0, nc.scalar)
        dup_k(0)
        dup_q(0)
        kq_chunk(1, nc.vector)
        dup_k(1)
        dup_q(1)
        kq_chunk(2, nc.scalar)
        dup_k(2)
        dup_q(2)
        v_chunk(0, nc.vector)

        # v chunks interleaved into iblock 0 (key: (ibi, g) -> emit fn)
        ib_chunks = {
            (0, 1): lambda: v_chunk(1, nc.vector),
            (0, 3): lambda: v_chunk(2, nc.vector),
            (0, 5): lambda: v_chunk(3, nc.vector),
            (0, 7): lambda: v_chunk(4, nc.vector),
            (0, 9): lambda: v_chunk(5, nc.vector),
        }

        # ---- main attention loop ----
        # PV queue lag 3 and carried across iblock boundaries: the PE queue
        # is in-order, so a PV emitted right after its exp stalls the score
        # matmuls behind it; with lag 3 the exp has long completed.
        def emit_pv(pv, pex, pg, iw):
            nc.tensor.matmul(
                pv[:, :iw],
                lhsT=v8_sb[:, pg, :, :],
                rhs=pex[:, :, :iw],
                start=(pg == 0),
                stop=(pg == NG - 1),
                perf_mode=DR,
            )

        def res_store(ibi, pv, ioff, iw):
            res_sb = resp.tile([DK + 1, 512], f16, tag="res", name="res_sb")
            # split halves across engines: DVE is the binding exp engine
            # (5x1223 + copies), so give ScalarE the first half
            h = iw // 2
            nc.scalar.copy(res_sb[:, :h], pv[: DK + 1, :h])
            nc.vector.tensor_copy(res_sb[:, h:iw], pv[: DK + 1, h:iw])
            nc.sync.dma_start(
                out=res_d[:, ioff : ioff + iw], in_=res_sb[:, :iw]
            )

        pvq = []  # [(ex, g, ibi)]
        state = {"T": None, "prev": None}  # prev: (ibi, pv, ioff, iw)

        def pop_pv():
            ex, g, ibi_ = pvq.pop(0)
            ioff_, iw_ = IBLOCKS[ibi_]
            emit_pv(state["T"], ex, g, iw_)

        for ibi, (ioff, iw) in enumerate(IBLOCKS):
            for g in range(NG):
                # pop PVs BEFORE this group's score matmuls: the PV
                # stream hides the next matmul's ldweights (in-order PE
                # queue + one-deep weight shadow slot)
                if g == 3 and ibi > 0:
                    pop_pv()  # PV(10, prev); pv(prev) now complete
                    pibi, ppv, pioff, piw = state["prev"]
                    res_store(pibi, ppv, pioff, piw)
                elif g == 4:
                    # double-buffered T: the new PV stream starts without
                    # waiting for the prev T's res drain
                    state["T"] = tp.tile([128, 512], f32, tag="T", name="T")
                    pop_pv()  # first PV of this iblock, into the new T
                elif len(pvq) >= 4:
                    pop_pv()
                if (ibi, g) in ib_chunks:
                    ib_chunks[(ibi, g)]()
                sc = scp.tile([128, 1024], f32, tag="sc", name="sc")
                sc3 = sc.rearrange("p (b w) -> p b w", b=2)[:, :, :iw]
                # score pair: even j-tile on array rows 0-63, odd j-tile on
                # rows 64-127 (tile_position from base_partition 64) -> the
                # two matmuls run concurrently in the PE array
                nc.tensor.matmul(
                    sc3[:, 0, :],
                    lhsT=k_lo[:, g * 128 : (g + 1) * 128],
                    rhs=qk_sb[0:DK, ioff : ioff + iw],
                    start=True,
                    stop=True,
                )
                nc.tensor.matmul(
                    sc3[:, 1, :],
                    lhsT=qk_sb[64:128, (2 * g + 1) * 128 : (2 * g + 2) * 128],
                    rhs=q_hi[64:128, ioff : ioff + iw],
                    start=True,
                    stop=True,
                )
                ex = expp.tile([128, 2, 512], f8, tag="ex", name="ex")
                # whole-group exp, alternating engines: even g ScalarE
                # (native exp), odd g DVE (Schraudolph byte encode)
                if g % 2 == 0:
                    nc.scalar.activation(
                        out=ex[:, :, :iw],
                        in_=sc3,
                        func=Exp,
                        bias=ebias_sb,
                        scale=0.125,
                    )
                else:
                    nc.vector.tensor_scalar(
                        ex[:, :, :iw].bitcast(u8), sc3,
                        SCHRAU_SCALE, SCHRAU_BIAS, Mult, Add,
                    )
                pvq.append((ex, g, ibi))
            state["prev"] = (ibi, state["T"], ioff, iw)
        while pvq:
            pop_pv()
        pibi, ppv, pioff, piw = state["prev"]
        res_store(pibi, ppv, pioff, piw)

    nc.compile()
    return nc


def _get_nc():
    global _NC
    if _NC is None:
        _NC = _build()
    return _NC


def _ones_tail():
    import ml_dtypes

    m = np.zeros((128, 1), dtype=ml_dtypes.float8_e4m3)
    m[:SVALID_LAST] = 1.0
    return m


def _make_in_maps(inputs):
    x = np.asarray(inputs["x"], dtype=np.float32)
    w_proj = np.asarray(inputs["w_proj"], dtype=np.float32)
    b_proj = np.asarray(inputs["b_proj"], dtype=np.float32)
    w_out = np.asarray(inputs["w_out"], dtype=np.float32)
    in_maps = []
    for core in range(8):
        b, h = divmod(core, H)
        base = h * 3 * DK
        in_maps.append(
            {
                "xT": np.ascontiguousarray(x[b].reshape(C, S).astype(np.float16)),
                "w3": np.ascontiguousarray(
                    w_proj[:, base : base + 3 * DK].astype(np.float16)
                ),
                "bqk2": np.ascontiguousarray(
                    np.concatenate(
                        [
                            b_proj[base : base + DK],
                            b_proj[base + DK : base + 2 * DK],
                        ]
                    ).astype(np.float32)[:, None]
                ),
                "ones_tail": _ones_tail(),
            }
        )
    return in_maps


def kernel(x, w_proj, b_proj, w_out, b_out):
    from concourse.bass_utils import run_bass_kernel_spmd

    x = np.asarray(x, dtype=np.float32)
    w_proj = np.asarray(w_proj, dtype=np.float32)
    b_proj = np.asarray(b_proj, dtype=np.float32)
    w_out = np.asarray(w_out, dtype=np.float32)
    b_out = np.asarray(b_out, dtype=np.float32)

    B = x.shape[0]
    nc = _get_nc()
    in_maps = _make_in_maps(
        {"x": x, "w_proj": w_proj, "b_proj": b_proj, "w_out": w_out, "b_out": b_out}
    )
    res = run_bass_kernel_spmd(nc, in_maps, list(range(8)))

    outs = np.zeros((B, C, S), dtype=np.float32)
    for b in range(B):
        acc = x[b].reshape(C, S).astype(np.float32) + b_out[:, None]
        for h in range(H):
            core = b * H + h
            r65 = res.results[core]["res"].astype(np.float32)  # [65, S]
            rn = r65[:DK] / r65[DK : DK + 1]  # softmax-normalized resT [64, S]
            woh = w_out[h * DK : (h + 1) * DK, :]  # [64, C]
            bv = b_proj[h * 3 * DK + 2 * DK : h * 3 * DK + 3 * DK]
            corr = bv @ woh  # [C]
            acc = acc + woh.T @ rn + corr[:, None]
        outs[b] = acc
    return outs.reshape(B, C, 14, 14, 14)
